# revision 1
# baseline (speedup 1.0000x reference)
"""CrossHeadAttention Trainium2 kernel (8-core SPMD, data+head parallel).

Reference computation (per batch b):
    k = x_enc @ Wk ; v = x_enc @ Wv ; q = x @ Wq        (bias-free linears)
    wei = softmax((q @ k^T) / sqrt(1024))  per head
    out = wei @ v                                        -> [B, T, H, D]

Sharding: 8 cores = 2 batches x 4 head-groups (4 heads each). Each core
receives x[b], x_enc[b] and the 256-column slice of Wq/Wk/Wv for its heads,
and produces out[b][:, :, hg*4:(hg+1)*4, :]. No cross-core communication.

Per-core dataflow (matmuls in float32r = full-rate ~fp32):
  x_enc --PE transpose--> xeT[c,s] --W-stationary matmul--> kT[d,s], vT[d,s]
  x     --PE transpose--> xT[c,t]  -----------------------> qT[d,t]
  vT --PE transpose--> v[s,d] (+ones column for softmax sums)
  S^T[s,t] = k q^T   (K=64 contraction, 2 heads row-packed via tile_position)
  P^T = exp(S^T / 32) on ScalarE (scores are ~N(0,1): no max-subtraction)
  outT[d_aug,t] = v_aug.T @ P^T  (psum-accumulated over s; row 64 = sums)
  out[t,d] = PE-transpose(outT) * 1/sums  (DVE), DMA to HBM.

The transposed activations are built in 512-column chunks that feed their
projections immediately and die, so SBUF holds one rotating 16 KiB/partition
chunk pool instead of 64 KiB static buffers. The kernel runs as two phases
with scoped PSUM pools: a projection phase (6-bank rotating psum; psum->sbuf
rounding copies split between DVE and the otherwise-idle ScalarE) and an
attention phase (4 banks score double-buffer + 2 PV accumulators + 2
finalize banks), with the exp activation table preloaded at t=0.
"""

from contextlib import ExitStack

import numpy as np

import concourse.bacc as bacc
import concourse.tile as tile
from concourse import mybir
from concourse.bass_utils import run_bass_kernel_spmd
from concourse.masks import make_identity

# Problem constants (hardcoded per spec)
B = 2
T = 2048          # query length
S = 2048          # key/value length
C = 1024          # n_embd
H = 16            # total heads
D = 64            # head size
N_CORES = 8
HG = H // (N_CORES // B)       # heads per core = 4
DCORE = HG * D                 # 256 projected dims per core
P = 128                        # partitions
CT = C // P                    # 8 contraction tiles
NPAIR = HG // 2                # 2 head pairs per core
TCH = 512                      # t-chunk width in attention
NTCH = T // TCH                # 4
ST = S // P                    # 16 s-tiles

F32 = mybir.dt.float32
F32R = mybir.dt.float32r
AF = mybir.ActivationFunctionType

SCALE = float(C) ** -0.5       # 1/32, folded into the exp activation


def _build_chain(nc, rows, aux, xtp, src_dram, projs, identity, rowtag):
    """Stream src[t, c] through PE-transpose into rotating [c, 512] chunks,
    and run every projection in `projs` on each chunk as soon as it lands.

    projs: list of (w_slice [P, CT, P] f32r, out_slice_fn(chunk_idx) -> AP).
    """
    for sch in range(src_dram.shape[0] // 512):
        _build_chain_chunk(nc, rows, aux, xtp, src_dram, projs, identity,
                           rowtag, sch, act_copies=True)


def _chain_chunk_pieces(nc, rows, aux, xtp, src_dram, projs, identity,
                        rowtag, sch, act_copies=False):
    """Emission pieces for one 512-wide x^T chunk + its projections.

    Returns a list of zero-arg callables; calling them in order (possibly
    interleaved with other emission) builds the chunk. When act_copies is
    set, half the psum->sbuf copies go to ScalarE instead of DVE (used
    pre-attention while ScalarE is otherwise idle).
    """
    state = {}

    def row_piece(r4):
        def go():
            if r4 == 0:
                state["xc"] = xtp.tile([P, CT, 512], F32R, tag="xch",
                                       name="xch")
            r = sch * 4 + r4
            row = rows.tile([P, C], F32, tag=rowtag, name="row")
            nc.sync.dma_start(out=row, in_=src_dram[r * P:(r + 1) * P, :])
            for cq in range(CT // 4):
                tp = aux.tile([P, 4 * P], F32, tag="aux", name="tp")
                for j in range(4):
                    ct = 4 * cq + j
                    nc.tensor.transpose(
                        tp[:, j * P:(j + 1) * P],
                        row[:, ct * P:(ct + 1) * P], identity)
                if act_copies and cq % 2:
                    copy_fn = nc.scalar.copy
                else:
                    copy_fn = lambda out, in_: nc.vector.tensor_copy(
                        out=out, in_=in_)
                copy_fn(
                    out=state["xc"][:, 4 * cq:4 * cq + 4,
                                    r4 * P:(r4 + 1) * P],
                    in_=tp.rearrange("p (j t) -> p j t", j=4))
        return go

    def proj_piece(w_slice, out_fn):
        def go():
            ps = aux.tile([P, 512], F32, tag="aux", name="ps")
            for ct in range(CT):
                nc.tensor.matmul(
                    ps, w_slice[:, ct, :], state["xc"][:, ct, :],
                    start=(ct == 0), stop=(ct == CT - 1))
            nc.vector.tensor_copy(out=out_fn(sch), in_=ps)
        return go

    return [row_piece(r4) for r4 in range(4)] +            [proj_piece(w, f) for w, f in projs]


def _build_chain_chunk(nc, rows, aux, xtp, src_dram, projs, identity,
                       rowtag, sch, act_copies=False):
    for piece in _chain_chunk_pieces(nc, rows, aux, xtp, src_dram, projs,
                                     identity, rowtag, sch, act_copies):
        piece()


def _build_v_transpose(nc, aux, vT, v_sb, identity, pt):
    """v_sb[s, 2pt:2pt+2, d] = (vT pair tile)^T via PE transpose."""
    for sq in range(ST // 4):
        tp = aux.tile([P, 4 * P], F32, tag="aux", name="tpv")
        for j in range(4):
            st = 4 * sq + j
            nc.tensor.transpose(
                tp[:, j * P:(j + 1) * P],
                vT.bitcast(F32)[:, st * P:(st + 1) * P], identity)
        for j in range(4):
            st = 4 * sq + j
            nc.vector.tensor_copy(
                out=v_sb[:, st, 2 * pt:2 * pt + 2, 0:D],
                in_=tp[:, j * P:(j + 1) * P].rearrange(
                    "p (h d) -> p h d", h=2))


def _build_attention_tch(nc, spsum, pvpools, aux, psb, otp, fin,
                         kT, qT, v_sb, identity, out, pair, tch,
                         interleave=()):
    """Attention st-loop for one head pair and one t-chunk -> oT tiles.

    `interleave`: emission pieces (e.g. next chunk's build) spliced between
    st iterations so the static schedule overlaps them with the exp stream.
    """
    if True:
        interleave = list(interleave)
        tsl = slice(tch * TCH, (tch + 1) * TCH)
        pv_ps = [pvpools[h2].tile([D + 1, TCH], F32, tag=f"pv{h2}",
                                  name=f"pv{h2}")
                 for h2 in range(2)]
        for st in range(ST):
            s_ps = spsum.tile([P, 2 * TCH], F32, tag="s", name="s_ps")
            for h2 in range(2):
                nc.tensor.matmul(
                    s_ps[:, h2 * TCH:(h2 + 1) * TCH],
                    kT[h2 * D:(h2 + 1) * D, pair, st * P:(st + 1) * P],
                    qT[h2 * D:(h2 + 1) * D, pair, tsl],
                    start=True, stop=True,
                    tile_position=(h2 * D, 0),
                )
            p_sb = psb.tile([P, 2 * TCH], F32R, tag="p", name="p_sb")
            nc.scalar.activation(out=p_sb, in_=s_ps, func=AF.Exp, scale=SCALE)
            for h2 in range(2):
                nc.tensor.matmul(
                    pv_ps[h2],
                    v_sb[:, st, 2 * pair + h2, :],
                    p_sb[:, h2 * TCH:(h2 + 1) * TCH],
                    start=(st == 0), stop=(st == ST - 1),
                )
            if interleave and st % 2 == 1:
                interleave.pop(0)()
        for piece in interleave:
            piece()
        oT = []
        for h2 in range(2):
            t_ = otp.tile([D + 1, TCH], F32, tag=f"oT{pair}{h2}",
                          name=f"oT{pair}{h2}")
            nc.vector.tensor_copy(out=t_, in_=pv_ps[h2])
            oT.append(t_)
        return oT


def _build_finalize_tch(nc, spsum, fin, oT, identity, out, pair, tch):
    """Transpose oT heads into a spsum bank, normalize by sums, store.

    Uses the spsum pool (not aux) so the next chunk-build's transposes are
    never serialized behind this tail work.
    """
    for sub in range(TCH // P):
        tt = tch * (TCH // P) + sub
        o_tile = fin.tile([P, 2 * D], F32, tag="o", name="o_tile")
        tp = spsum.tile([P, 2 * (D + 1)], F32, tag="ft", name="ft")
        for h2 in range(2):
            nc.tensor.transpose(
                tp[:, h2 * (D + 1):(h2 + 1) * (D + 1)],
                oT[h2][:, sub * P:(sub + 1) * P],
                identity[0:D + 1, 0:D + 1])
        tph = tp.rearrange("p (h e) -> p h e", h=2)
        r2 = fin.tile([P, 2], F32, tag="r", name="r2")
        nc.vector.reciprocal(out=r2, in_=tph[:, :, D])
        for h2 in range(2):
            nc.vector.tensor_scalar_mul(
                out=o_tile[:, h2 * D:(h2 + 1) * D],
                in0=tph[:, h2, 0:D], scalar1=r2[:, h2:h2 + 1])
        # SWDGE: keeps this dependent store out of SP's in-order
        # stream so it cannot head-of-line-block later row loads
        nc.gpsimd.dma_start(
            out=out[tt * P:(tt + 1) * P,
                    pair * 2 * D:(pair + 1) * 2 * D],
            in_=o_tile)


def _attention_phase(nc, tc, kT, qT, v_sb, identity, out,
                     psb, otp, fin):
    with tc.tile_pool(name="spsum", bufs=2, space="PSUM") as spsum, \
         tc.tile_pool(name="pvpsum0", bufs=1, space="PSUM") as pvp0, \
         tc.tile_pool(name="pvpsum1", bufs=1, space="PSUM") as pvp1, \
         tc.tile_pool(name="ftpsum", bufs=2, space="PSUM") as ftp:
        pvpools = (pvp0, pvp1)
        for tch in range(NTCH):
            oT0 = _build_attention_tch(
                nc, spsum, pvpools, None, psb, otp, fin,
                kT, qT, v_sb, identity, out, 0, tch)
            # pair-0 finalize emitted before pair-1 attention so its
            # transposes/stores run under pair-1's exp stream
            _build_finalize_tch(nc, ftp, fin, oT0, identity, out, 0, tch)
            oT1 = _build_attention_tch(
                nc, spsum, pvpools, None, psb, otp, fin,
                kT, qT, v_sb, identity, out, 1, tch)
            _build_finalize_tch(nc, ftp, fin, oT1, identity, out, 1, tch)


def _build_body(nc, tc, x, xe, wq, wk, wv, out):
    with ExitStack() as ctx:
        consts = ctx.enter_context(tc.tile_pool(name="consts", bufs=1))
        big = ctx.enter_context(tc.tile_pool(name="big", bufs=1))
        psb = ctx.enter_context(tc.tile_pool(name="psb", bufs=3))
        otp = ctx.enter_context(tc.tile_pool(name="otp", bufs=2))
        fin = ctx.enter_context(tc.tile_pool(name="fin", bufs=3))

        identity = consts.tile([P, P], F32)
        make_identity(nc, identity)
        # prime the ScalarE exp table at t=0 so the ~2.7us ACT_TABLE_LOAD is
        # off the critical path of the first real exp
        dummy = consts.tile([1, 2], F32)
        nc.vector.memset(dummy, 0.0)
        nc.scalar.activation(out=dummy, in_=dummy, func=AF.Exp)

        kT = big.tile([P, NPAIR, S], F32R, tag="kT")
        qT = big.tile([P, NPAIR, T], F32R, tag="qT")
        vT0 = big.tile([P, S], F32R, tag="vT0")
        vT1 = big.tile([P, S], F32R, tag="vT1")
        # v, with a ones column appended per head (col D) for softmax sums
        v_sb = big.tile([P, ST, HG, D + 1], F32R, tag="v_sb")
        nc.vector.memset(v_sb[:, :, :, D].bitcast(F32), 1.0)

        with tc.tile_pool(name="xtp", bufs=2) as xtp, \
             tc.tile_pool(name="rows", bufs=3) as rows, \
             tc.tile_pool(name="wpool", bufs=1) as wpool:

            # weights: DMA f32 staging -> DVE rounding copy -> f32r
            w_sbs = {}
            for name, wdram in (("wk", wk), ("wv", wv), ("wq", wq)):
                stage = wpool.tile([P, CT, DCORE], F32, tag="wstage",
                                   name="wstage")
                nc.gpsimd.dma_start(
                    out=stage, in_=wdram.rearrange("(ct p) d -> p ct d", p=P))
                wsb = wpool.tile([P, CT, DCORE], F32R, tag=f"{name}_sb",
                                 name=f"{name}_sb")
                nc.vector.tensor_copy(out=wsb, in_=stage)
                w_sbs[name] = wsb

            def _dsl(wname, dt_):
                return w_sbs[wname][:, :, dt_ * P:(dt_ + 1) * P]

            with tc.tile_pool(name="chainps", bufs=6, space="PSUM") as aux:
                # xe chain: k^T and v^T for both pairs, chunk-streamed
                _build_chain(
                    nc, rows, aux, xtp, xe,
                    [(_dsl("wk", 0),
                      lambda s: kT[:, 0, s * 512:(s + 1) * 512]),
                     (_dsl("wv", 0),
                      lambda s: vT0[:, s * 512:(s + 1) * 512]),
                     (_dsl("wk", 1),
                      lambda s: kT[:, 1, s * 512:(s + 1) * 512]),
                     (_dsl("wv", 1),
                      lambda s: vT1[:, s * 512:(s + 1) * 512])],
                    identity, "row")
                _build_v_transpose(nc, aux, vT0, v_sb, identity, 0)
                _build_v_transpose(nc, aux, vT1, v_sb, identity, 1)

                # x chain: q^T for both pairs
                qproj = [(_dsl("wq", 0),
                          lambda s: qT[:, 0, s * 512:(s + 1) * 512]),
                         (_dsl("wq", 1),
                          lambda s: qT[:, 1, s * 512:(s + 1) * 512])]
                _build_chain(nc, rows, aux, xtp, x, qproj, identity, "row")

            _attention_phase(nc, tc, kT, qT, v_sb, identity, out,
                             psb, otp, fin)


def build_program():
    nc = bacc.Bacc("TRN2", target_bir_lowering=False, debug=False,
                   num_devices=N_CORES)

    x = nc.dram_tensor("x", [T, C], F32, kind="ExternalInput").ap()
    xe = nc.dram_tensor("xe", [S, C], F32, kind="ExternalInput").ap()
    wq = nc.dram_tensor("wq", [C, DCORE], F32, kind="ExternalInput").ap()
    wk = nc.dram_tensor("wk", [C, DCORE], F32, kind="ExternalInput").ap()
    wv = nc.dram_tensor("wv", [C, DCORE], F32, kind="ExternalInput").ap()
    out = nc.dram_tensor("out", [T, DCORE], F32, kind="ExternalOutput").ap()

    with tile.TileContext(nc) as tc:
        _build_body(nc, tc, x, xe, wq, wk, wv, out)
    nc.compile()
    return nc


_NC_CACHE = None


def _get_program():
    global _NC_CACHE
    if _NC_CACHE is None:
        _NC_CACHE = build_program()
    return _NC_CACHE


def kernel(x_enc, x, Wk, Wq, Wv):
    x_enc = np.asarray(x_enc, dtype=np.float32)
    x = np.asarray(x, dtype=np.float32)
    Wk = np.asarray(Wk, dtype=np.float32)
    Wq = np.asarray(Wq, dtype=np.float32)
    Wv = np.asarray(Wv, dtype=np.float32)

    nc = _get_program()
    in_maps = []
    for core in range(N_CORES):
        b, hg = divmod(core, N_CORES // B)
        csl = slice(hg * DCORE, (hg + 1) * DCORE)
        in_maps.append({
            "x": np.ascontiguousarray(x[b]),
            "xe": np.ascontiguousarray(x_enc[b]),
            "wq": np.ascontiguousarray(Wq[:, csl]),
            "wk": np.ascontiguousarray(Wk[:, csl]),
            "wv": np.ascontiguousarray(Wv[:, csl]),
        })
    res = run_bass_kernel_spmd(nc, in_maps, list(range(N_CORES)))

    full = np.empty((B, T, H, D), dtype=np.float32)
    for core in range(N_CORES):
        b, hg = divmod(core, N_CORES // B)
        o = res.results[core]["out"].reshape(T, HG, D)
        full[b, :, hg * HG:(hg + 1) * HG, :] = o
    return full



# revision 6
# speedup vs baseline: 1.2953x; 1.2953x over previous
"""CrossHeadAttention Trainium2 kernel (8-core SPMD, data+head parallel).

Reference computation (per batch b):
    k = x_enc @ Wk ; v = x_enc @ Wv ; q = x @ Wq        (bias-free linears)
    wei = softmax((q @ k^T) / sqrt(1024))  per head
    out = wei @ v                                        -> [B, T, H, D]

Sharding: 8 cores = 2 batches x 4 head-groups (4 heads each). Each core
receives x[b], x_enc[b] and the 256-column slice of Wq/Wk/Wv for its heads,
and produces out[b][:, :, hg*4:(hg+1)*4, :]. No cross-core communication.

v2 design (vs the PE-transpose/f32r v1):
  * All inputs are converted to bf16 on the host. This halves HBM traffic
    and enables the XBAR DMA-transpose path (2-byte dtypes only), which
    loads x^T and xe^T straight into SBUF with zero PE/DVE work.
  * Projections: Wk/Wq stationary -> k^T,q^T [d,*] directly; xe^T-chunk
    stationary with Wv moving -> v in natural [s,d] layout (PV needs it).
  * Scores S^T[s,t] per (tchunk, head-pair, s-tile), two heads packed in
    the PE via tile_position at K=64. Moving operand is bf16 qT (1 cyc/col).
  * exp on ScalarE ([128,1024] per instruction, scale=1/32 folded in,
    bf16 output) -> P^T in SBUF.
  * PV with P^T as the *stationary* operand and v (plus a ones column for
    the softmax denominators) moving: out[t, h*65+{d,sum}] accumulates in
    PSUM over the 16 s-tiles. Output lands in natural [t,d] order - no
    v transpose, no output transpose.
  * DVE normalizes (reciprocal + per-partition scalar mul) into an SBUF
    f32 staging tile; one DMA store per t-chunk.
  * The q projection for chunk i+1 is woven one matmul at a time between
    st iterations of chunk i so the ScalarE exp stream (the critical
    path) never waits on PE.
"""

from contextlib import ExitStack

import numpy as np
import ml_dtypes

import concourse.bacc as bacc
import concourse.tile as tile
from concourse import mybir
from concourse.bass_utils import run_bass_kernel_spmd

# Problem constants (hardcoded per spec)
B = 2
T = 2048          # query length
S = 2048          # key/value length
C = 1024          # n_embd
H = 16            # total heads
D = 64            # head size
N_CORES = 8
HG = H // (N_CORES // B)       # heads per core = 4
DCORE = HG * D                 # 256 projected dims per core
P = 128                        # partitions
CT = C // P                    # 8 contraction tiles
NPAIR = HG // 2                # 2 head pairs per core
TCH = 512                      # t-chunk width in attention
NTCH = T // TCH                # 4
ST = S // P                    # 16 s-tiles
DE = D + 1                     # head slot width in PV psum (denominator col)

F32 = mybir.dt.float32
BF16 = mybir.dt.bfloat16
AF = mybir.ActivationFunctionType

SCALE = float(C) ** -0.5       # 1/32, folded into the exp activation


def _emit_transposes(nc, dst, src_dram, n_rows):
    """XBAR DMA-transpose src[n_rows, C] (bf16) into dst [P, CT, n_rows].

    Row-half 0 for every ct first, so consumers of the first n_rows/2
    columns unblock after 8 transfers instead of 15.
    """
    half = n_rows // 2
    for h in range(2):
        for ct in range(CT):
            nc.sync.dma_start_transpose(
                dst[:, ct, h * half:(h + 1) * half],
                src_dram[h * half:(h + 1) * half, ct * P:(ct + 1) * P])


def _proj_kq_pieces(nc, kqps, copy_eng, w_sb, xT, dstT, chunk):
    """One 512-wide chunk of a k/q projection: out^T[d, chunk] for both
    128-row d-halves. Returns zero-arg pieces, one PE matmul (or one
    psum->sbuf copy) each, so they can be woven into other PE streams."""
    pieces = []
    for half in range(2):
        state = {}

        def mm(half=half, ct=None, state=state):
            if ct == 0:
                state["ps"] = kqps.tile([P, TCH], F32, tag="kq", name="kq_ps")
            nc.tensor.matmul(
                state["ps"], w_sb[:, ct, half * P:(half + 1) * P],
                xT[:, ct, chunk * TCH:(chunk + 1) * TCH],
                start=(ct == 0), stop=(ct == CT - 1))

        def cp(half=half, state=state):
            out = dstT[:, half, chunk * TCH:(chunk + 1) * TCH]
            if copy_eng == "scalar":
                nc.scalar.copy(out=out, in_=state["ps"])
            else:
                nc.vector.tensor_copy(out=out, in_=state["ps"])

        for ct in range(CT):
            pieces.append((lambda f=mm, ct=ct: f(ct=ct)))
        pieces.append(cp)
    return pieces


def _proj_v(nc, vps, w_sb, xeT, v_sb):
    """v[s, d] for all 4 heads: xe^T s-tile stationary, Wv moving.

    Full-bank [P, 512] psum tiles: matmul start=True resets the whole
    2 KiB bank, so a half-bank tile sharing a bank with its double-buffer
    partner would be wiped mid-accumulation.
    """
    for st in range(ST):
        ps = vps.tile([P, 2 * DCORE], F32, tag="v", name="v_ps")
        for ct in range(CT):
            nc.tensor.matmul(
                ps[:, 0:DCORE], xeT[:, ct, st * P:(st + 1) * P],
                w_sb[:, ct, :],
                start=(ct == 0), stop=(ct == CT - 1))
        nc.vector.tensor_copy(
            out=v_sb[:, st, :, 0:D],
            in_=ps[:, 0:DCORE].rearrange("p (h d) -> p h d", h=HG))


def _attention(nc, tc, kT, qT, v_sb, out_sb, out_dram, psb, fin, qproj_work):
    """Attention over 4 t-chunks x 2 head pairs, st-streamed.

    qproj_work: zero-arg pieces (later q-projection chunks) woven between
    st iterations so PE fill-work hides under the ScalarE exp stream.
    """
    with tc.tile_pool(name="spsum", bufs=2, space="PSUM") as spsum, \
         tc.tile_pool(name="pvpsA", bufs=1, space="PSUM") as pvpsA, \
         tc.tile_pool(name="pvpsB", bufs=1, space="PSUM") as pvpsB:
        for tch in range(NTCH):
            for pair in range(NPAIR):
                tsl = slice(tch * TCH, (tch + 1) * TCH)
                # two 1-bank pv tiles: t-subtiles (0,1) and (2,3). Four
                # accumulation groups share each bank, so they must run
                # start=False onto a zeroed bank: matmul start=True resets
                # the WHOLE bank, clobbering sibling groups.
                pvs = [pvpsA.tile([P, 2, 2 * DE], F32, tag="pvA", name="pvA"),
                       pvpsB.tile([P, 2, 2 * DE], F32, tag="pvB", name="pvB")]
                nc.vector.memset(pvs[0], 0.0)
                nc.vector.memset(pvs[1], 0.0)
                for st in range(ST):
                    s_ps = spsum.tile([P, 2 * TCH], F32, tag="s", name="s_ps")
                    for h2 in range(2):
                        nc.tensor.matmul(
                            s_ps[:, h2 * TCH:(h2 + 1) * TCH],
                            kT[h2 * D:(h2 + 1) * D, pair,
                               st * P:(st + 1) * P],
                            qT[h2 * D:(h2 + 1) * D, pair, tsl],
                            start=True, stop=True,
                            tile_position=(h2 * D, 0),
                        )
                    p_sb = psb.tile([P, 2 * TCH], BF16, tag="p", name="p_sb")
                    nc.scalar.activation(out=p_sb, in_=s_ps,
                                         func=AF.Exp, scale=SCALE)
                    for h2 in range(2):
                        for sub in range(TCH // P):
                            nc.tensor.matmul(
                                pvs[sub // 2][:, sub % 2,
                                              h2 * DE:(h2 + 1) * DE],
                                p_sb[:, h2 * TCH + sub * P:
                                     h2 * TCH + (sub + 1) * P],
                                v_sb[:, st, 2 * pair + h2, :],
                                start=False, stop=(st == ST - 1),
                                skip_group_check=True,
                            )
                    if qproj_work and st % 2 == 1:
                        qproj_work.pop(0)()
                # normalize this pair's 512 t rows into the f32 staging tile
                for sub in range(TCH // P):
                    pv = pvs[sub // 2]
                    pvh = pv[:, sub % 2, :].rearrange("p (h e) -> p h e", h=2)
                    r2 = fin.tile([P, 2], F32, tag="r", name="r2")
                    nc.vector.reciprocal(out=r2, in_=pvh[:, :, D])
                    for h2 in range(2):
                        nc.vector.tensor_scalar_mul(
                            out=out_sb[:, tch * (TCH // P) + sub,
                                       (2 * pair + h2) * D:
                                       (2 * pair + h2 + 1) * D],
                            in0=pvh[:, h2, 0:D],
                            scalar1=r2[:, h2:h2 + 1])
            # store this t-chunk (one HWDGE inst per chunk)
            c0 = tch * (TCH // P)
            nc.sync.dma_start(
                out=out_dram.rearrange("(s p) d -> p s d", p=P)[
                    :, c0:c0 + TCH // P, :],
                in_=out_sb[:, c0:c0 + TCH // P, :])
        for piece in qproj_work:
            piece()


def _build_body(nc, tc, x, xe, wq, wk, wv, out):
    with ExitStack() as ctx:
        consts = ctx.enter_context(tc.tile_pool(name="consts", bufs=1))
        big = ctx.enter_context(tc.tile_pool(name="big", bufs=1))
        psb = ctx.enter_context(tc.tile_pool(name="psb", bufs=3))
        fin = ctx.enter_context(tc.tile_pool(name="fin", bufs=3))

        # prime the ScalarE exp table at t=0 so the ACT_TABLE_LOAD is off
        # the critical path of the first real exp
        dummy = consts.tile([1, 2], F32)
        nc.vector.memset(dummy, 0.0)
        nc.scalar.activation(out=dummy, in_=dummy, func=AF.Exp)

        # transposed activations (bf16), filled by XBAR DMA transpose
        xeT = big.tile([P, CT, S], BF16, tag="xeT")
        xT = big.tile([P, CT, T], BF16, tag="xT")
        # projected tensors
        kT = big.tile([P, NPAIR, S], BF16, tag="kT")   # [2 heads x 64d, pair, s]
        qT = big.tile([P, NPAIR, T], BF16, tag="qT")
        v_sb = big.tile([P, ST, HG, DE], BF16, tag="v_sb")  # ones col at D
        nc.vector.memset(v_sb[:, :, :, D], 1.0)
        out_sb = big.tile([P, T // P, DCORE], F32, tag="out_sb")

        # weights (bf16, small) + input transposes; xe first so k/v start asap
        wpool = ctx.enter_context(tc.tile_pool(name="wpool", bufs=1))
        w_sbs = {}
        for name, wdram in (("wk", wk), ("wv", wv), ("wq", wq)):
            wsb = wpool.tile([P, CT, DCORE], BF16, tag=f"{name}_sb",
                             name=f"{name}_sb")
            nc.sync.dma_start(
                out=wsb, in_=wdram.rearrange("(ct p) d -> p ct d", p=P))
            w_sbs[name] = wsb
        _emit_transposes(nc, xeT, xe, S)
        _emit_transposes(nc, xT, x, T)

        with tc.tile_pool(name="kqps", bufs=2, space="PSUM") as kqps, \
             tc.tile_pool(name="vps", bufs=2, space="PSUM") as vps:
            # k then v then q chunk 0; psum->sbuf copies ride ScalarE/DVE
            # (both idle before the exp stream starts)
            for chunk in range(S // TCH):
                for piece in _proj_kq_pieces(
                        nc, kqps, "scalar", w_sbs["wk"], xeT, kT, chunk):
                    piece()
            _proj_v(nc, vps, w_sbs["wv"], xeT, v_sb)
            for piece in _proj_kq_pieces(
                    nc, kqps, "scalar", w_sbs["wq"], xT, qT, 0):
                piece()

        with tc.tile_pool(name="qps", bufs=1, space="PSUM") as qps:
            qproj_work = []
            for chunk in range(1, T // TCH):
                qproj_work += _proj_kq_pieces(
                    nc, qps, "vector", w_sbs["wq"], xT, qT, chunk)
            _attention(nc, tc, kT, qT, v_sb, out_sb, out, psb, fin,
                       qproj_work)


def build_program():
    nc = bacc.Bacc("TRN2", target_bir_lowering=False, debug=False,
                   num_devices=N_CORES)

    x = nc.dram_tensor("x", [T, C], BF16, kind="ExternalInput").ap()
    xe = nc.dram_tensor("xe", [S, C], BF16, kind="ExternalInput").ap()
    wq = nc.dram_tensor("wq", [C, DCORE], BF16, kind="ExternalInput").ap()
    wk = nc.dram_tensor("wk", [C, DCORE], BF16, kind="ExternalInput").ap()
    wv = nc.dram_tensor("wv", [C, DCORE], BF16, kind="ExternalInput").ap()
    out = nc.dram_tensor("out", [T, DCORE], F32, kind="ExternalOutput").ap()

    with tile.TileContext(nc) as tc:
        _build_body(nc, tc, x, xe, wq, wk, wv, out)
    nc.compile()
    return nc


_NC_CACHE = None


def _get_program():
    global _NC_CACHE
    if _NC_CACHE is None:
        _NC_CACHE = build_program()
    return _NC_CACHE


def kernel(x_enc, x, Wk, Wq, Wv):
    bf16 = ml_dtypes.bfloat16
    x_enc = np.asarray(x_enc, dtype=np.float32).astype(bf16)
    x = np.asarray(x, dtype=np.float32).astype(bf16)
    Wk = np.asarray(Wk, dtype=np.float32).astype(bf16)
    Wq = np.asarray(Wq, dtype=np.float32).astype(bf16)
    Wv = np.asarray(Wv, dtype=np.float32).astype(bf16)

    nc = _get_program()
    in_maps = []
    for core in range(N_CORES):
        b, hg = divmod(core, N_CORES // B)
        csl = slice(hg * DCORE, (hg + 1) * DCORE)
        in_maps.append({
            "x": np.ascontiguousarray(x[b]),
            "xe": np.ascontiguousarray(x_enc[b]),
            "wq": np.ascontiguousarray(Wq[:, csl]),
            "wk": np.ascontiguousarray(Wk[:, csl]),
            "wv": np.ascontiguousarray(Wv[:, csl]),
        })
    res = run_bass_kernel_spmd(nc, in_maps, list(range(N_CORES)))

    full = np.empty((B, T, H, D), dtype=np.float32)
    for core in range(N_CORES):
        b, hg = divmod(core, N_CORES // B)
        o = res.results[core]["out"].reshape(T, HG, D)
        full[b, :, hg * HG:(hg + 1) * HG, :] = o
    return full
